# revision 10
# baseline (speedup 1.0000x reference)
"""JacobiKANLinear TRN2 Bass kernel.

out = silu(x) @ W_base^T + einsum('bik,oik->bo', P(tanh(x)), C) + bias

Host-side algebra: Jacobi polynomials (A=B=1, degree 5) are re-expressed in
the monomial basis.  D[o,i,j] = sum_k C[o,i,k] * T[k,j] where T holds the
monomial coefficients of P_k.  The j=0 term is constant (t^0 == 1) and folds
into the bias.  The device computes 6 feature blocks
[silu(x), t, t^2, t^3, t^4, t^5] (t = tanh(x)) and one fused matmul with
contraction 6*1024 = 6144.

Sharding (8 cores): 4 batch groups x 2 out-feature halves.  Per core:
batch shard 2048 rows, out shard 512 cols.

Matmuls run in bf16.  The t-powers are computed in f32 (t_f, t2f, t3f
temps) and rounded once to bf16 (the bf16 t block is a second direct
tanh so the DVE chain stays short).  Accumulation is f32 in PSUM.
Measured rel-err ~1.1e-2 vs the 2e-2 gate.

Startup schedule: every phase-1 input rides ONE HWDGE ring (SP) in a
strict order chosen so bytes land exactly when consumed: x0 quarters
interleaved with kt0/kt1, then growing weight batches with x1 (halved),
x2, x3 and the bias slotted between them.  A single ring serializes the
SDMA FIFO, so the first-needed tiles are not bandwidth-starved by
later-needed ones (parallel rings round-robin at packet granularity and
delay everything).  Phase-1 matmuls are emitted in explicit readiness
order of (chunk, block) 8-matmul granules -- an in-order PE queue must
never hold a granule whose operands arrive later than a ready one.
Warm-up matmuls fed from vector-memset tiles start the PE p-state ramp
at context entry.  Phase 2 (chunks 4-15) is chunk-major with block prep
(x DMA on the ACT ring + activations + DVE powers) emitted two chunks
ahead.  Bias is a precomputed [128,512] broadcast folded into the
PSUM->SBUF copy.
"""
import numpy as np
import ml_dtypes

import concourse.bass as bass
import concourse.mybir as mybir
import concourse.tile as tile
from concourse import bacc
from concourse.bass_utils import run_bass_kernel_spmd

BATCH = 8192
IN_F = 1024
OUT_F = 1024
DEGREE = 5
A = 1.0
B = 1.0

N_CORES = 8
BATCH_GROUPS = 4
OUT_HALVES = 2
B_SHARD = BATCH // BATCH_GROUPS        # 2048
O_SHARD = OUT_F // OUT_HALVES          # 512
N_BLOCKS = DEGREE + 1                  # 6 feature blocks
N_KT = N_BLOCKS * IN_F // 128          # 48 contraction tiles of 128
N_CHUNKS = B_SHARD // 128              # 16 batch chunks per core
IT_PER_BLOCK = IN_F // 128             # 8 in-feature tiles per block

P1 = 4                                 # chunks processed during weight load
SETS = 5                               # rotating feature-block tag sets

# Phase-1 (chunk, block) granule emission order: sorted by the time the
# granule's operands (activation chain x weight-tile arrival) are ready.
P1_ORDER = [
    (0, 0), (1, 0), (0, 1), (1, 1), (2, 0), (0, 2), (1, 2), (2, 1),
    (2, 2), (3, 0), (3, 1), (0, 3), (1, 3), (2, 3), (3, 2), (3, 3),
    (0, 4), (1, 4), (2, 4), (3, 4), (0, 5), (1, 5), (2, 5), (3, 5),
]

F32 = mybir.dt.float32
BF16 = mybir.dt.bfloat16
SILU = mybir.ActivationFunctionType.Silu
TANH = mybir.ActivationFunctionType.Tanh


def _jacobi_monomial_matrix():
    """T[k, j] = coefficient of t^j in P_k (A=B=1), float64."""
    T = np.zeros((DEGREE + 1, DEGREE + 1), dtype=np.float64)
    polys = [np.zeros(DEGREE + 1) for _ in range(DEGREE + 1)]
    polys[0][0] = 1.0
    if DEGREE >= 1:
        polys[1][1] = A + 1.0
        polys[1][0] = 0.5 * (A - B)
    for k in range(2, DEGREE + 1):
        alpha_n = 2.0 * k * (k + A + B) * (2 * k + A + B - 2)
        beta_n = (2 * k + A + B - 1) * (A ** 2 - B ** 2)
        gamma_n = (2 * k + A + B - 2) * (2 * k + A + B - 1) * (2 * k + A + B)
        delta_n = 2.0 * (k + A - 1) * (k + B - 1) * (2 * k + A + B)
        p = np.zeros(DEGREE + 1)
        p += (beta_n / gamma_n) * polys[k - 1]
        p[1:] += (alpha_n / gamma_n) * polys[k - 1][:-1]
        p -= (delta_n / gamma_n) * polys[k - 2]
        polys[k] = p
    for k in range(DEGREE + 1):
        T[k] = polys[k]
    return T


def _build_nc():
    nc = bacc.Bacc()
    xt_in = nc.declare_dram_parameter(
        "xt", [N_CHUNKS, 128, IT_PER_BLOCK, 128], F32, isOutput=False)
    w_in = nc.declare_dram_parameter(
        "w", [128, N_KT, O_SHARD], BF16, isOutput=False)
    biasbc_in = nc.declare_dram_parameter(
        "biasbc", [128, O_SHARD], F32, isOutput=False)
    out = nc.declare_dram_parameter("out", [B_SHARD, O_SHARD], F32, isOutput=True)

    with tile.TileContext(nc) as tc:
        with tc.tile_pool(name="wpool", bufs=1) as wpool, \
             tc.tile_pool(name="bpool", bufs=1) as bpool, \
             tc.tile_pool(name="tpool", bufs=4) as tpool, \
             tc.tile_pool(name="xpool", bufs=4) as xpool, \
             tc.tile_pool(name="opool", bufs=3) as opool, \
             tc.tile_pool(name="psum", bufs=1, space="PSUM") as psum_pool:

            w_sb = wpool.tile([128, N_KT, O_SHARD], BF16)
            bias_bc = wpool.tile([128, O_SHARD], F32)
            warm_l = wpool.tile([1, 128], BF16)
            warm_r = wpool.tile([1, O_SHARD], BF16)

            # PE warm-up fillers fed from vector-memset tiles: the DVE is
            # idle at context entry, so the PE goes busy immediately and
            # the p-state ramps before the first real matmul.
            nc.vector.memset(warm_l[:].bitcast(mybir.dt.uint32), 0)
            nc.vector.memset(warm_r[:].bitcast(mybir.dt.uint32), 0)
            # 9 matmuls ~= 4.3us of continuous PE-busy: enough to complete a
            # full HAM activity window and unthrottle the clock to 2.4 GHz
            # before the first real matmul.  (6 was ~3.2us -- just under the
            # ~3.4us window, and the whole phase 1 ran at 1.2 GHz.)  The
            # extra warm-up time is absorbed by the early weight-DMA waits,
            # and once warm, those short (<2us) stalls never re-throttle.
            warm_ps = psum_pool.tile([128, O_SHARD], F32, tag="warm")
            for _ in range(9):
                nc.tensor.matmul(
                    warm_ps[:], warm_l[:], warm_r[:], start=True, stop=True)

            x_tiles = []
            for c in range(P1):
                x_c = xpool.tile([128, IT_PER_BLOCK, 128], F32, tag="x",
                                 name=f"x_{c}")
                x_tiles.append(x_c)

            # THE single-ring input stream (SP HWDGE, strict FIFO): order
            # is chosen so each transfer lands just before its consumer.
            q = IT_PER_BLOCK // 4                          # 2 it tiles
            h = IT_PER_BLOCK // 2                          # 4 it tiles
            nc.sync.dma_start(out=x_tiles[0][:, :q, :], in_=xt_in[0][:, :q, :])
            nc.sync.dma_start(out=w_sb[:, 0:1, :], in_=w_in[:, 0:1, :])
            nc.sync.dma_start(out=w_sb[:, 1:2, :], in_=w_in[:, 1:2, :])
            nc.sync.dma_start(out=x_tiles[0][:, q:h, :], in_=xt_in[0][:, q:h, :])
            nc.sync.dma_start(out=w_sb[:, 2:4, :], in_=w_in[:, 2:4, :])
            nc.sync.dma_start(out=x_tiles[0][:, h:, :], in_=xt_in[0][:, h:, :])
            nc.sync.dma_start(out=w_sb[:, 4:8, :], in_=w_in[:, 4:8, :])
            nc.sync.dma_start(out=x_tiles[1][:, :h, :], in_=xt_in[1][:, :h, :])
            nc.sync.dma_start(out=x_tiles[1][:, h:, :], in_=xt_in[1][:, h:, :])
            nc.sync.dma_start(out=w_sb[:, 8:12, :], in_=w_in[:, 8:12, :])
            nc.sync.dma_start(out=w_sb[:, 12:16, :], in_=w_in[:, 12:16, :])
            nc.sync.dma_start(out=x_tiles[2][:], in_=xt_in[2])
            nc.sync.dma_start(out=w_sb[:, 16:20, :], in_=w_in[:, 16:20, :])
            nc.sync.dma_start(out=w_sb[:, 20:24, :], in_=w_in[:, 20:24, :])
            nc.sync.dma_start(out=x_tiles[3][:], in_=xt_in[3])
            nc.sync.dma_start(out=bias_bc[:], in_=biasbc_in[:])
            nc.sync.dma_start(out=w_sb[:, 24:32, :], in_=w_in[:, 24:32, :])
            nc.sync.dma_start(out=w_sb[:, 32:40, :], in_=w_in[:, 32:40, :])
            nc.sync.dma_start(out=w_sb[:, 40:48, :], in_=w_in[:, 40:48, :])

            def alloc_set(s):
                # bf16 matmul blocks.  t4 later reuses the silu slot, t5
                # the t slot (writes ordered between the slots' consumers).
                a_t = bpool.tile([128, IT_PER_BLOCK, 128], BF16,
                                 tag=f"A{s}", name=f"blkA{s}")
                b_t = bpool.tile([128, IT_PER_BLOCK, 128], BF16,
                                 tag=f"B{s}", name=f"blkB{s}")
                t2b = bpool.tile([128, IT_PER_BLOCK, 128], BF16,
                                 tag=f"C{s}", name=f"blkC{s}")
                t3b = bpool.tile([128, IT_PER_BLOCK, 128], BF16,
                                 tag=f"D{s}", name=f"blkD{s}")
                return a_t, b_t, t2b, t3b

            def alloc_tmp():
                # f32 power-chain temps: one rounding per bf16 block.
                t_f = tpool.tile([128, IT_PER_BLOCK, 128], F32, tag="tf",
                                 name="t_f")
                t2f = tpool.tile([128, IT_PER_BLOCK, 128], F32, tag="t2f",
                                 name="t2f")
                t3f = tpool.tile([128, IT_PER_BLOCK, 128], F32, tag="t3f",
                                 name="t3f")
                return t_f, t2f, t3f

            def emit_powers_dve(blks, tmps):
                # t^2, t^3 in f32 with one bf16 rounding each.
                _, b_t, t2b, t3b = blks
                t_f, t2f, t3f = tmps
                nc.vector.tensor_mul(t2f[:], t_f[:], t_f[:])
                nc.vector.tensor_copy(t2b[:], t2f[:])
                nc.vector.tensor_mul(t3f[:], t2f[:], t_f[:])
                nc.vector.tensor_copy(t3b[:], t3f[:])

            def finish_chunk(m, ps):
                o_m = opool.tile([128, O_SHARD], F32, tag="o", name=f"o_{m}")
                bsl = bass.ts(m, 128)
                if m == N_CHUNKS - 1:
                    # Last chunk: halve the epilogue so the second half's
                    # add overlaps the first half's DMA, split across the
                    # otherwise-idle Act ring and the SP ring.
                    oh = O_SHARD // 2
                    nc.vector.tensor_add(
                        o_m[:, :oh], ps[:, :oh], bias_bc[:, :oh])
                    nc.scalar.dma_start(
                        out=out[bsl, :oh], in_=o_m[:, :oh])
                    nc.vector.tensor_add(
                        o_m[:, oh:], ps[:, oh:], bias_bc[:, oh:])
                    nc.sync.dma_start(
                        out=out[bsl, oh:], in_=o_m[:, oh:])
                else:
                    nc.vector.tensor_add(o_m[:], ps[:], bias_bc[:])
                    nc.sync.dma_start(out=out[bsl, :], in_=o_m[:])

            # Phase-1 activations.  ACT queue (in-order) emission matches
            # x arrival: x0 in quarters/half, x1 in halves, x2/x3 whole.
            # The bf16 t block is a second direct tanh (not a DVE cast of
            # the f32 one): the DVE chain is phase 1's scarce resource.
            blocks1 = [alloc_set(c) for c in range(P1)]
            tmps1 = [alloc_tmp() for c in range(P1)]
            nc.scalar.activation(blocks1[0][0][:, :q, :],
                                 x_tiles[0][:, :q, :], SILU)
            nc.scalar.activation(blocks1[0][0][:, q:h, :],
                                 x_tiles[0][:, q:h, :], SILU)
            nc.scalar.activation(blocks1[0][0][:, h:, :],
                                 x_tiles[0][:, h:, :], SILU)
            nc.scalar.activation(blocks1[0][1][:], x_tiles[0][:], TANH)
            nc.scalar.activation(tmps1[0][0][:], x_tiles[0][:], TANH)
            nc.scalar.activation(blocks1[1][0][:, :h, :],
                                 x_tiles[1][:, :h, :], SILU)
            nc.scalar.activation(blocks1[1][0][:, h:, :],
                                 x_tiles[1][:, h:, :], SILU)
            nc.scalar.activation(blocks1[1][1][:], x_tiles[1][:], TANH)
            nc.scalar.activation(tmps1[1][0][:], x_tiles[1][:], TANH)
            nc.scalar.activation(blocks1[2][0][:], x_tiles[2][:], SILU)
            nc.scalar.activation(blocks1[2][1][:], x_tiles[2][:], TANH)
            nc.scalar.activation(tmps1[2][0][:], x_tiles[2][:], TANH)
            nc.scalar.activation(blocks1[3][0][:], x_tiles[3][:], SILU)
            nc.scalar.activation(blocks1[3][1][:], x_tiles[3][:], TANH)
            nc.scalar.activation(tmps1[3][0][:], x_tiles[3][:], TANH)
            # DVE power chain, ordered by first consumption: every chunk's
            # t^2 before any chunk's t^3 (granule order consumes squares
            # across chunks before cubes).
            for c in range(P1):
                t_f, t2f, _ = tmps1[c]
                nc.vector.tensor_mul(t2f[:], t_f[:], t_f[:])
                nc.vector.tensor_copy(blocks1[c][2][:], t2f[:])
            for c in range(P1):
                t_f, t2f, t3f = tmps1[c]
                nc.vector.tensor_mul(t3f[:], t2f[:], t_f[:])
                nc.vector.tensor_copy(blocks1[c][3][:], t3f[:])

            ps1 = [psum_pool.tile([128, O_SHARD], F32, tag="ps", bufs=P1 + 1,
                                  name=f"ps1_{c}") for c in range(P1)]

            # Phase-1 t^4/t^5 also ride the DVE (not gpsimd): concurrent
            # DVE+gpsimd SBUF traffic arbitrates for the shared port pair
            # and roughly halves both.  Emission points sit after the
            # granule whose PE reads clear the overwritten slot, so they
            # land at the DVE queue tail, after the critical casts.
            dve_after = {
                2: [(0, 4)],            # t4_0 after (0,1)
                3: [(0, 5), (1, 4)],    # t5_0, t4_1 after (1,1)
                6: [(1, 5)],            # t5_1 after (1,2)
                7: [(2, 4)],            # t4_2 after (2,1)
                8: [(2, 5)],            # t5_2 after (2,2)
                10: [(3, 4), (3, 5)],   # t4_3, t5_3 after (3,1)
            }
            for pos, (c, b) in enumerate(P1_ORDER):
                a_t, b_t, t2b, t3b = blocks1[c]
                src = (a_t, b_t, t2b, t3b, a_t, b_t)[b]
                for it in range(IT_PER_BLOCK):
                    kt = b * IT_PER_BLOCK + it
                    nc.tensor.matmul(
                        ps1[c][:], src[:, it, :], w_sb[:, kt, :],
                        start=(kt == 0), stop=(kt == N_KT - 1))
                for (gc, gb) in dve_after.get(pos, ()):
                    ga, gb_t, _, _ = blocks1[gc]
                    _, gt2f, gt3f = tmps1[gc]
                    if gb == 4:
                        nc.vector.tensor_mul(ga[:], gt2f[:], gt2f[:])
                    else:
                        nc.vector.tensor_mul(gb_t[:], gt2f[:], gt3f[:])

            # Phase 2: remaining chunks, chunk-major (weights resident).
            # Block prep (x DMA on the ACT ring + activations + DVE
            # powers) is emitted two chunks ahead.
            def prep(m):
                x_m = xpool.tile([128, IT_PER_BLOCK, 128], F32, tag="x",
                                 name=f"x_{m}")
                nc.scalar.dma_start(out=x_m[:], in_=xt_in[m])
                blks = alloc_set(m % SETS)
                tmps = alloc_tmp()
                nc.scalar.activation(blks[0][:], x_m[:], SILU)
                nc.scalar.activation(blks[1][:], x_m[:], TANH)
                nc.scalar.activation(tmps[0][:], x_m[:], TANH)
                emit_powers_dve(blks, tmps)
                return blks, tmps

            prepped = {}
            for m in range(P1, min(P1 + 2, N_CHUNKS)):
                prepped[m] = prep(m)
            for c in range(P1):
                finish_chunk(c, ps1[c])

            for m in range(P1, N_CHUNKS):
                (a_t, b_t, t2b, t3b), (t_f, t2f, t3f) = prepped.pop(m)
                ps = psum_pool.tile([128, O_SHARD], F32, tag="ps", bufs=P1 + 1,
                                    name=f"ps_{m}")
                for b in range(N_BLOCKS):
                    if b == 4:
                        nc.gpsimd.tensor_mul(a_t[:], t2f[:], t2f[:])
                    elif b == 5:
                        nc.gpsimd.tensor_mul(b_t[:], t2f[:], t3f[:])
                    for it in range(IT_PER_BLOCK):
                        kt = b * IT_PER_BLOCK + it
                        src = (a_t, b_t, t2b, t3b, a_t, b_t)[b]
                        nc.tensor.matmul(
                            ps[:], src[:, it, :], w_sb[:, kt, :],
                            start=(kt == 0), stop=(kt == N_KT - 1))
                if m + 2 < N_CHUNKS:
                    prepped[m + 2] = prep(m + 2)
                finish_chunk(m, ps)
    nc.finalize()
    return nc


_NC_CACHE = None


def _get_nc():
    global _NC_CACHE
    if _NC_CACHE is None:
        _NC_CACHE = _build_nc()
    return _NC_CACHE


def _prepare_host(x, base_weight, jacobi_coeffs, bias):
    T = _jacobi_monomial_matrix()
    D = np.einsum("oik,kj->oij", jacobi_coeffs.astype(np.float64), T)
    bias_eff = bias.astype(np.float64) + D[:, :, 0].sum(axis=1)

    # W'[f, o]: 6 blocks of IN_F feature rows: silu -> base_weight, t^j -> D_j
    w_full = np.empty((N_BLOCKS * IN_F, OUT_F), dtype=np.float32)
    w_full[0:IN_F] = base_weight.T
    for j in range(1, N_BLOCKS):
        w_full[j * IN_F:(j + 1) * IN_F] = D[:, :, j].T.astype(np.float32)

    w_halves = []
    bias_halves = []
    for hh in range(OUT_HALVES):
        wh = w_full[:, hh * O_SHARD:(hh + 1) * O_SHARD]
        # SBUF layout [128, N_KT, O_SHARD]: [p, kt, n] = wh[kt*128 + p, n]
        wh = np.ascontiguousarray(
            wh.reshape(N_KT, 128, O_SHARD).transpose(1, 0, 2)
            .astype(ml_dtypes.bfloat16))
        w_halves.append(wh)
        bh = bias_eff[hh * O_SHARD:(hh + 1) * O_SHARD].astype(np.float32)
        bias_halves.append(
            np.ascontiguousarray(np.broadcast_to(bh[None, :], (128, O_SHARD))))

    xt_groups = []
    for g in range(BATCH_GROUPS):
        xs = x[g * B_SHARD:(g + 1) * B_SHARD]              # (B_SHARD, IN_F)
        # [c, p, it, b] = xs[c*128 + b, it*128 + p]
        xt = np.ascontiguousarray(
            xs.reshape(N_CHUNKS, 128, IT_PER_BLOCK, 128).transpose(0, 3, 2, 1))
        xt_groups.append(xt)
    return xt_groups, w_halves, bias_halves


def kernel(x, base_weight, jacobi_coeffs, bias):
    x = np.asarray(x, dtype=np.float32)
    base_weight = np.asarray(base_weight, dtype=np.float32)
    jacobi_coeffs = np.asarray(jacobi_coeffs, dtype=np.float32)
    bias = np.asarray(bias, dtype=np.float32)

    xt_groups, w_halves, bias_halves = _prepare_host(
        x, base_weight, jacobi_coeffs, bias)

    in_maps = []
    for c in range(N_CORES):
        g, hh = c // OUT_HALVES, c % OUT_HALVES
        in_maps.append({
            "xt": xt_groups[g],
            "w": w_halves[hh],
            "biasbc": bias_halves[hh],
        })

    nc = _get_nc()
    res = run_bass_kernel_spmd(nc, in_maps, core_ids=list(range(N_CORES)))

    out = np.empty((BATCH, OUT_F), dtype=np.float32)
    for c in range(N_CORES):
        g, hh = c // OUT_HALVES, c % OUT_HALVES
        out[g * B_SHARD:(g + 1) * B_SHARD,
            hh * O_SHARD:(hh + 1) * O_SHARD] = res.results[c]["out"]
    return out


# revision 11
# speedup vs baseline: 1.1865x; 1.1865x over previous
"""JacobiKANLinear TRN2 Bass kernel.

out = silu(x) @ W_base^T + einsum('bik,oik->bo', P(tanh(x)), C) + bias

Host-side algebra: Jacobi polynomials (A=B=1, degree 5) are re-expressed in
the monomial basis.  D[o,i,j] = sum_k C[o,i,k] * T[k,j] where T holds the
monomial coefficients of P_k.  The j=0 term is constant (t^0 == 1) and folds
into the bias.  The device computes 6 feature blocks
[silu(x), t, t^2, t^3, t^4, t^5] (t = tanh(x)) and one fused matmul with
contraction 6*1024 = 6144.

Sharding (8 cores): 4 batch groups x 2 out-feature halves.  Per core:
batch shard 2048 rows, out shard 512 cols.

Matmuls run in bf16.  The t-powers are computed in f32 (t_f, t2f, t3f
temps) and rounded once to bf16 (the bf16 t block is a second direct
tanh so the DVE chain stays short).  Accumulation is f32 in PSUM.
Measured rel-err ~1.1e-2 vs the 2e-2 gate.

Startup schedule: every phase-1 input rides ONE HWDGE ring (SP) in a
strict order chosen so bytes land exactly when consumed: x0 quarters
interleaved with kt0/kt1, then growing weight batches with x1 (halved),
x2, x3 and the bias slotted between them.  A single ring serializes the
SDMA FIFO, so the first-needed tiles are not bandwidth-starved by
later-needed ones (parallel rings round-robin at packet granularity and
delay everything).  Phase-1 matmuls are emitted in explicit readiness
order of (chunk, block) 8-matmul granules -- an in-order PE queue must
never hold a granule whose operands arrive later than a ready one.
Warm-up matmuls fed from vector-memset tiles start the PE p-state ramp
at context entry.  Phase 2 (chunks 4-15) is chunk-major with block prep
(x DMA on the ACT ring + activations + DVE powers) emitted two chunks
ahead.  Bias is a precomputed [128,512] broadcast folded into the
PSUM->SBUF copy.
"""
import numpy as np
import ml_dtypes

import concourse.bass as bass
import concourse.mybir as mybir
import concourse.tile as tile
from concourse import bacc
from concourse.bass_utils import run_bass_kernel_spmd

BATCH = 8192
IN_F = 1024
OUT_F = 1024
DEGREE = 5
A = 1.0
B = 1.0

N_CORES = 8
BATCH_GROUPS = 4
OUT_HALVES = 2
B_SHARD = BATCH // BATCH_GROUPS        # 2048
O_SHARD = OUT_F // OUT_HALVES          # 512
N_BLOCKS = DEGREE + 1                  # 6 feature blocks
N_KT = N_BLOCKS * IN_F // 128          # 48 contraction tiles of 128
N_CHUNKS = B_SHARD // 128              # 16 batch chunks per core
IT_PER_BLOCK = IN_F // 128             # 8 in-feature tiles per block

P1 = 4                                 # chunks processed during weight load
SETS = 5                               # rotating feature-block tag sets

# Phase-1 (chunk, block) granule emission order: sorted by the time the
# granule's operands (activation chain x weight-tile arrival) are ready.
P1_ORDER = [
    (0, 0), (1, 0), (0, 1), (1, 1), (2, 0), (0, 2), (1, 2), (2, 1),
    (2, 2), (3, 0), (3, 1), (0, 3), (1, 3), (2, 3), (3, 2), (3, 3),
    (0, 4), (1, 4), (2, 4), (3, 4), (0, 5), (1, 5), (2, 5), (3, 5),
]

F32 = mybir.dt.float32
BF16 = mybir.dt.bfloat16
SILU = mybir.ActivationFunctionType.Silu
TANH = mybir.ActivationFunctionType.Tanh


def _jacobi_monomial_matrix():
    """T[k, j] = coefficient of t^j in P_k (A=B=1), float64."""
    T = np.zeros((DEGREE + 1, DEGREE + 1), dtype=np.float64)
    polys = [np.zeros(DEGREE + 1) for _ in range(DEGREE + 1)]
    polys[0][0] = 1.0
    if DEGREE >= 1:
        polys[1][1] = A + 1.0
        polys[1][0] = 0.5 * (A - B)
    for k in range(2, DEGREE + 1):
        alpha_n = 2.0 * k * (k + A + B) * (2 * k + A + B - 2)
        beta_n = (2 * k + A + B - 1) * (A ** 2 - B ** 2)
        gamma_n = (2 * k + A + B - 2) * (2 * k + A + B - 1) * (2 * k + A + B)
        delta_n = 2.0 * (k + A - 1) * (k + B - 1) * (2 * k + A + B)
        p = np.zeros(DEGREE + 1)
        p += (beta_n / gamma_n) * polys[k - 1]
        p[1:] += (alpha_n / gamma_n) * polys[k - 1][:-1]
        p -= (delta_n / gamma_n) * polys[k - 2]
        polys[k] = p
    for k in range(DEGREE + 1):
        T[k] = polys[k]
    return T


def _build_nc():
    nc = bacc.Bacc()
    xt_in = nc.declare_dram_parameter(
        "xt", [N_CHUNKS, 128, IT_PER_BLOCK, 128], F32, isOutput=False)
    w_in = nc.declare_dram_parameter(
        "w", [128, N_KT, O_SHARD], BF16, isOutput=False)
    biasbc_in = nc.declare_dram_parameter(
        "biasbc", [128, O_SHARD], F32, isOutput=False)
    out = nc.declare_dram_parameter("out", [B_SHARD, O_SHARD], F32, isOutput=True)

    with tile.TileContext(nc) as tc:
        with tc.tile_pool(name="wpool", bufs=1) as wpool, \
             tc.tile_pool(name="bpool", bufs=1) as bpool, \
             tc.tile_pool(name="tpool", bufs=4) as tpool, \
             tc.tile_pool(name="xpool", bufs=4) as xpool, \
             tc.tile_pool(name="opool", bufs=3) as opool, \
             tc.tile_pool(name="psum", bufs=1, space="PSUM") as psum_pool:

            w_sb = wpool.tile([128, N_KT, O_SHARD], BF16)
            bias_bc = wpool.tile([128, O_SHARD], F32)
            warm_l = wpool.tile([1, 128], BF16)
            warm_r = wpool.tile([1, O_SHARD], BF16)

            # PE warm-up fillers fed from vector-memset tiles: the DVE is
            # idle at context entry, so the PE goes busy immediately and
            # the p-state ramps before the first real matmul.
            nc.vector.memset(warm_l[:].bitcast(mybir.dt.uint32), 0)
            nc.vector.memset(warm_r[:].bitcast(mybir.dt.uint32), 0)
            warm_ps = psum_pool.tile([128, O_SHARD], F32, tag="warm")
            for _ in range(6):
                nc.tensor.matmul(
                    warm_ps[:], warm_l[:], warm_r[:], start=True, stop=True)

            x_tiles = []
            for c in range(P1):
                x_c = xpool.tile([128, IT_PER_BLOCK, 128], F32, tag="x",
                                 name=f"x_{c}")
                x_tiles.append(x_c)

            # THE single-ring input stream (SP HWDGE, strict FIFO): order
            # is chosen so each transfer lands just before its consumer.
            q = IT_PER_BLOCK // 4                          # 2 it tiles
            h = IT_PER_BLOCK // 2                          # 4 it tiles
            nc.sync.dma_start(out=x_tiles[0][:, :q, :], in_=xt_in[0][:, :q, :])
            nc.sync.dma_start(out=w_sb[:, 0:1, :], in_=w_in[:, 0:1, :])
            nc.sync.dma_start(out=w_sb[:, 1:2, :], in_=w_in[:, 1:2, :])
            nc.sync.dma_start(out=x_tiles[0][:, q:h, :], in_=xt_in[0][:, q:h, :])
            nc.sync.dma_start(out=w_sb[:, 2:4, :], in_=w_in[:, 2:4, :])
            nc.sync.dma_start(out=x_tiles[0][:, h:, :], in_=xt_in[0][:, h:, :])
            nc.sync.dma_start(out=w_sb[:, 4:8, :], in_=w_in[:, 4:8, :])
            nc.sync.dma_start(out=x_tiles[1][:, :h, :], in_=xt_in[1][:, :h, :])
            nc.sync.dma_start(out=x_tiles[1][:, h:, :], in_=xt_in[1][:, h:, :])
            nc.sync.dma_start(out=w_sb[:, 8:12, :], in_=w_in[:, 8:12, :])
            nc.sync.dma_start(out=w_sb[:, 12:16, :], in_=w_in[:, 12:16, :])
            nc.sync.dma_start(out=x_tiles[2][:], in_=xt_in[2])
            nc.sync.dma_start(out=w_sb[:, 16:20, :], in_=w_in[:, 16:20, :])
            nc.sync.dma_start(out=w_sb[:, 20:24, :], in_=w_in[:, 20:24, :])
            nc.sync.dma_start(out=x_tiles[3][:], in_=xt_in[3])
            nc.sync.dma_start(out=bias_bc[:], in_=biasbc_in[:])
            nc.sync.dma_start(out=w_sb[:, 24:32, :], in_=w_in[:, 24:32, :])
            nc.sync.dma_start(out=w_sb[:, 32:40, :], in_=w_in[:, 32:40, :])
            nc.sync.dma_start(out=w_sb[:, 40:48, :], in_=w_in[:, 40:48, :])

            def alloc_set(s):
                # bf16 matmul blocks.  t4 later reuses the silu slot, t5
                # the t slot (writes ordered between the slots' consumers).
                a_t = bpool.tile([128, IT_PER_BLOCK, 128], BF16,
                                 tag=f"A{s}", name=f"blkA{s}")
                b_t = bpool.tile([128, IT_PER_BLOCK, 128], BF16,
                                 tag=f"B{s}", name=f"blkB{s}")
                t2b = bpool.tile([128, IT_PER_BLOCK, 128], BF16,
                                 tag=f"C{s}", name=f"blkC{s}")
                t3b = bpool.tile([128, IT_PER_BLOCK, 128], BF16,
                                 tag=f"D{s}", name=f"blkD{s}")
                return a_t, b_t, t2b, t3b

            def alloc_tmp():
                # f32 power-chain temps: one rounding per bf16 block.
                t_f = tpool.tile([128, IT_PER_BLOCK, 128], F32, tag="tf",
                                 name="t_f")
                t2f = tpool.tile([128, IT_PER_BLOCK, 128], F32, tag="t2f",
                                 name="t2f")
                t3f = tpool.tile([128, IT_PER_BLOCK, 128], F32, tag="t3f",
                                 name="t3f")
                return t_f, t2f, t3f

            def emit_powers_dve(blks, tmps):
                # t^2, t^3 in f32 with one bf16 rounding each.
                _, b_t, t2b, t3b = blks
                t_f, t2f, t3f = tmps
                nc.vector.tensor_mul(t2f[:], t_f[:], t_f[:])
                nc.vector.tensor_copy(t2b[:], t2f[:])
                nc.vector.tensor_mul(t3f[:], t2f[:], t_f[:])
                nc.vector.tensor_copy(t3b[:], t3f[:])

            def finish_chunk(m, ps):
                o_m = opool.tile([128, O_SHARD], F32, tag="o", name=f"o_{m}")
                bsl = bass.ts(m, 128)
                if m == N_CHUNKS - 1:
                    # Last chunk: halve the epilogue so the second half's
                    # add overlaps the first half's DMA, split across the
                    # otherwise-idle Act ring and the SP ring.
                    oh = O_SHARD // 2
                    nc.vector.tensor_add(
                        o_m[:, :oh], ps[:, :oh], bias_bc[:, :oh])
                    nc.scalar.dma_start(
                        out=out[bsl, :oh], in_=o_m[:, :oh])
                    nc.vector.tensor_add(
                        o_m[:, oh:], ps[:, oh:], bias_bc[:, oh:])
                    nc.sync.dma_start(
                        out=out[bsl, oh:], in_=o_m[:, oh:])
                else:
                    nc.vector.tensor_add(o_m[:], ps[:], bias_bc[:])
                    nc.sync.dma_start(out=out[bsl, :], in_=o_m[:])

            # Phase-1 activations.  ACT queue (in-order) emission matches
            # x arrival: x0 in quarters/half, x1 in halves, x2/x3 whole.
            # The bf16 t block is a second direct tanh (not a DVE cast of
            # the f32 one): the DVE chain is phase 1's scarce resource.
            blocks1 = [alloc_set(c) for c in range(P1)]
            tmps1 = [alloc_tmp() for c in range(P1)]
            nc.scalar.activation(blocks1[0][0][:, :q, :],
                                 x_tiles[0][:, :q, :], SILU)
            nc.scalar.activation(blocks1[0][0][:, q:h, :],
                                 x_tiles[0][:, q:h, :], SILU)
            nc.scalar.activation(blocks1[0][0][:, h:, :],
                                 x_tiles[0][:, h:, :], SILU)
            nc.scalar.activation(blocks1[0][1][:], x_tiles[0][:], TANH)
            nc.scalar.activation(tmps1[0][0][:], x_tiles[0][:], TANH)
            nc.scalar.activation(blocks1[1][0][:, :h, :],
                                 x_tiles[1][:, :h, :], SILU)
            nc.scalar.activation(blocks1[1][0][:, h:, :],
                                 x_tiles[1][:, h:, :], SILU)
            nc.scalar.activation(blocks1[1][1][:], x_tiles[1][:], TANH)
            nc.scalar.activation(tmps1[1][0][:], x_tiles[1][:], TANH)
            nc.scalar.activation(blocks1[2][0][:], x_tiles[2][:], SILU)
            nc.scalar.activation(blocks1[2][1][:], x_tiles[2][:], TANH)
            nc.scalar.activation(tmps1[2][0][:], x_tiles[2][:], TANH)
            nc.scalar.activation(blocks1[3][0][:], x_tiles[3][:], SILU)
            nc.scalar.activation(blocks1[3][1][:], x_tiles[3][:], TANH)
            nc.scalar.activation(tmps1[3][0][:], x_tiles[3][:], TANH)
            # DVE power chain, ordered by first consumption: every chunk's
            # t^2 before any chunk's t^3 (granule order consumes squares
            # across chunks before cubes).
            for c in range(P1):
                t_f, t2f, _ = tmps1[c]
                nc.vector.tensor_mul(t2f[:], t_f[:], t_f[:])
                nc.vector.tensor_copy(blocks1[c][2][:], t2f[:])
            for c in range(P1):
                t_f, t2f, t3f = tmps1[c]
                nc.vector.tensor_mul(t3f[:], t2f[:], t_f[:])
                nc.vector.tensor_copy(blocks1[c][3][:], t3f[:])

            ps1 = [psum_pool.tile([128, O_SHARD], F32, tag="ps", bufs=P1 + 1,
                                  name=f"ps1_{c}") for c in range(P1)]

            # Phase-1 t^4/t^5 also ride the DVE (not gpsimd): concurrent
            # DVE+gpsimd SBUF traffic arbitrates for the shared port pair
            # and roughly halves both.  Emission points sit after the
            # granule whose PE reads clear the overwritten slot, so they
            # land at the DVE queue tail, after the critical casts.
            dve_after = {
                2: [(0, 4)],            # t4_0 after (0,1)
                3: [(0, 5), (1, 4)],    # t5_0, t4_1 after (1,1)
                6: [(1, 5)],            # t5_1 after (1,2)
                7: [(2, 4)],            # t4_2 after (2,1)
                8: [(2, 5)],            # t5_2 after (2,2)
                10: [(3, 4), (3, 5)],   # t4_3, t5_3 after (3,1)
            }
            for pos, (c, b) in enumerate(P1_ORDER):
                a_t, b_t, t2b, t3b = blocks1[c]
                src = (a_t, b_t, t2b, t3b, a_t, b_t)[b]
                for it in range(IT_PER_BLOCK):
                    kt = b * IT_PER_BLOCK + it
                    nc.tensor.matmul(
                        ps1[c][:], src[:, it, :], w_sb[:, kt, :],
                        start=(kt == 0), stop=(kt == N_KT - 1))
                for (gc, gb) in dve_after.get(pos, ()):
                    ga, gb_t, _, _ = blocks1[gc]
                    _, gt2f, gt3f = tmps1[gc]
                    if gb == 4:
                        nc.vector.tensor_mul(ga[:], gt2f[:], gt2f[:])
                    else:
                        nc.vector.tensor_mul(gb_t[:], gt2f[:], gt3f[:])

            # Phase 2: remaining chunks, chunk-major (weights resident).
            # Block prep (x DMA on the ACT ring + activations + DVE
            # powers) is emitted two chunks ahead.
            def prep(m):
                x_m = xpool.tile([128, IT_PER_BLOCK, 128], F32, tag="x",
                                 name=f"x_{m}")
                nc.scalar.dma_start(out=x_m[:], in_=xt_in[m])
                blks = alloc_set(m % SETS)
                tmps = alloc_tmp()
                nc.scalar.activation(blks[0][:], x_m[:], SILU)
                nc.scalar.activation(blks[1][:], x_m[:], TANH)
                nc.scalar.activation(tmps[0][:], x_m[:], TANH)
                emit_powers_dve(blks, tmps)
                return blks, tmps

            prepped = {}
            for m in range(P1, min(P1 + 2, N_CHUNKS)):
                prepped[m] = prep(m)
            for c in range(P1):
                finish_chunk(c, ps1[c])

            for m in range(P1, N_CHUNKS):
                (a_t, b_t, t2b, t3b), (t_f, t2f, t3f) = prepped.pop(m)
                ps = psum_pool.tile([128, O_SHARD], F32, tag="ps", bufs=P1 + 1,
                                    name=f"ps_{m}")
                for b in range(N_BLOCKS):
                    if b == 4:
                        nc.gpsimd.tensor_mul(a_t[:], t2f[:], t2f[:])
                    elif b == 5:
                        nc.gpsimd.tensor_mul(b_t[:], t2f[:], t3f[:])
                    for it in range(IT_PER_BLOCK):
                        kt = b * IT_PER_BLOCK + it
                        src = (a_t, b_t, t2b, t3b, a_t, b_t)[b]
                        nc.tensor.matmul(
                            ps[:], src[:, it, :], w_sb[:, kt, :],
                            start=(kt == 0), stop=(kt == N_KT - 1))
                if m + 2 < N_CHUNKS:
                    prepped[m + 2] = prep(m + 2)
                finish_chunk(m, ps)
    nc.finalize()
    return nc


_NC_CACHE = None


def _get_nc():
    global _NC_CACHE
    if _NC_CACHE is None:
        _NC_CACHE = _build_nc()
    return _NC_CACHE


def _prepare_host(x, base_weight, jacobi_coeffs, bias):
    T = _jacobi_monomial_matrix()
    D = np.einsum("oik,kj->oij", jacobi_coeffs.astype(np.float64), T)
    bias_eff = bias.astype(np.float64) + D[:, :, 0].sum(axis=1)

    # W'[f, o]: 6 blocks of IN_F feature rows: silu -> base_weight, t^j -> D_j
    w_full = np.empty((N_BLOCKS * IN_F, OUT_F), dtype=np.float32)
    w_full[0:IN_F] = base_weight.T
    for j in range(1, N_BLOCKS):
        w_full[j * IN_F:(j + 1) * IN_F] = D[:, :, j].T.astype(np.float32)

    w_halves = []
    bias_halves = []
    for hh in range(OUT_HALVES):
        wh = w_full[:, hh * O_SHARD:(hh + 1) * O_SHARD]
        # SBUF layout [128, N_KT, O_SHARD]: [p, kt, n] = wh[kt*128 + p, n]
        wh = np.ascontiguousarray(
            wh.reshape(N_KT, 128, O_SHARD).transpose(1, 0, 2)
            .astype(ml_dtypes.bfloat16))
        w_halves.append(wh)
        bh = bias_eff[hh * O_SHARD:(hh + 1) * O_SHARD].astype(np.float32)
        bias_halves.append(
            np.ascontiguousarray(np.broadcast_to(bh[None, :], (128, O_SHARD))))

    xt_groups = []
    for g in range(BATCH_GROUPS):
        xs = x[g * B_SHARD:(g + 1) * B_SHARD]              # (B_SHARD, IN_F)
        # [c, p, it, b] = xs[c*128 + b, it*128 + p]
        xt = np.ascontiguousarray(
            xs.reshape(N_CHUNKS, 128, IT_PER_BLOCK, 128).transpose(0, 3, 2, 1))
        xt_groups.append(xt)
    return xt_groups, w_halves, bias_halves


def kernel(x, base_weight, jacobi_coeffs, bias):
    x = np.asarray(x, dtype=np.float32)
    base_weight = np.asarray(base_weight, dtype=np.float32)
    jacobi_coeffs = np.asarray(jacobi_coeffs, dtype=np.float32)
    bias = np.asarray(bias, dtype=np.float32)

    xt_groups, w_halves, bias_halves = _prepare_host(
        x, base_weight, jacobi_coeffs, bias)

    in_maps = []
    for c in range(N_CORES):
        g, hh = c // OUT_HALVES, c % OUT_HALVES
        in_maps.append({
            "xt": xt_groups[g],
            "w": w_halves[hh],
            "biasbc": bias_halves[hh],
        })

    nc = _get_nc()
    res = run_bass_kernel_spmd(nc, in_maps, core_ids=list(range(N_CORES)))

    out = np.empty((BATCH, OUT_F), dtype=np.float32)
    for c in range(N_CORES):
        g, hh = c // OUT_HALVES, c % OUT_HALVES
        out[g * B_SHARD:(g + 1) * B_SHARD,
            hh * O_SHARD:(hh + 1) * O_SHARD] = res.results[c]["out"]
    return out
